# revision 42
# baseline (speedup 1.0000x reference)
"""Causal self-attention with RoPE, tensor-parallel over 8 NeuronCores.

Problem shapes: x [2, 2048, 2048], w_qkv [6144, 2048], w_out [2048, 2048],
H=16 heads, dh=128. Core c owns heads {2c, 2c+1}.

Per-core plan (all layouts chosen so no on-device transposes are needed):
  - host passes xT [B, D, L] plus per-core transposed weight shards
  - Q^T/K^T projection: one PSUM bank holds both heads side by side
    (psum[dh, 2*tok]); W^T-tile stationary x xT moving.  V is projected
    straight into natural [tok, dh] layout by flipping the operands.
  - RoPE during PSUM->SBUF copyback in 4 DVE ops per (w, chunk): the
    half-swap sign is folded into the host sin table ([-sin; +sin]) and
    cos/sin tables are pre-duplicated per head so both heads are processed
    in single ops; intermediates and q/k outputs are bf16 (DVE 2x modes).
  - scores^T[k, q] = K^T-tile (stationary, bf16) x Q^T (moving, bf16),
    running LAG=2 tiles ahead of the PV matmuls so the exp latency hides;
    exp on ACT with the 1/sqrt(dh) scale folded in, output bf16; on
    diagonal blocks the fully-masked columns are skipped in scores/exp
    (zeroed by idle gpsimd) and only a [128,128] triangle is mask-multiplied
  - unnormalized out^T[dh, q] accumulates V-tile x E^T (bf16); softmax
    denominator accumulates ones^T x E^T in the same pass
  - normalization: reciprocal of the [1, 512] sums row, broadcast across
    partitions with gpsimd partition_broadcast, multiplied in during the
    attention-out copyback (output bf16)
  - w_out partial[tok, e] = attnout^T-tile (stationary) x woT (bf16
    moving), summed over this core's heads in PSUM; written to DRAM as
    bf16; host sums the 8 partials in f32
  - cross-batch software pipeline: emission order is qkv(b0), attn(b0),
    qkv(b1), wout(b0), attn(b1) with wout(b1) interleaved one q-chunk
    behind, so the PE never waits on softmax normalization or batch seams

Matmuls run in float32r for the QKV projection (full PE rate for free
dim >= 256) and bf16 for scores/PV/wout (same PE rate, half the SBUF/DMA).
"""

import numpy as np
import ml_dtypes

import concourse.bass as bass
import concourse.mybir as mybir
import concourse.tile as tile
from concourse import bacc, library_config
from concourse.bass_utils import run_bass_kernel_spmd

B, L, D, H = 2, 2048, 2048, 16
DH = D // H  # 128
NCORES = 8
HPC = H // NCORES  # heads per core
ROPE_BASE = 10000.0
SCALE = 1.0 / float(np.sqrt(np.float32(DH)))

TOKC = 256  # token chunk width in the QKV projection phase
NCHUNK = L // TOKC  # 8
QC = 512  # q chunk width in the attention phase
NQC = L // QC  # 4
KT = L // 128  # 16 k tiles per sequence
KD = D // 128  # 16 contraction chunks for the projections

F32 = mybir.dt.float32
F32R = mybir.dt.float32r
BF16 = mybir.dt.bfloat16
POOL = {"xtp": 2, "tmps": 3, "esb": 8, "bcp": 4, "attn": 12, "outp": 10,
        "psA": 4}
LAG = 3  # scores-to-PV software pipeline depth (hides exp latency)
AF = mybir.ActivationFunctionType
ALU = mybir.AluOpType


def _body(nc, tc, aps, phases=("qkv", "attn", "wout")):
    xt, wq, wk, wv, wo, cs, sn, mk, out = aps
    with (
        tc.tile_pool(name="const", bufs=1) as const,
        tc.tile_pool(name="xtp", bufs=POOL["xtp"]) as xtp,
        tc.tile_pool(name="qkv", bufs=2) as qkvp,
        tc.tile_pool(name="tmps", bufs=POOL["tmps"]) as tmps,
        tc.tile_pool(name="esb", bufs=POOL["esb"]) as esbp,
        tc.tile_pool(name="bcp", bufs=POOL["bcp"]) as bcp,
        tc.tile_pool(name="attn", bufs=POOL["attn"]) as attnp,
        tc.tile_pool(name="outp", bufs=POOL["outp"]) as outp,
        tc.tile_pool(name="psA", bufs=POOL["psA"], space="PSUM") as psA,
        tc.tile_pool(name="psO", bufs=2, space="PSUM") as psO,
        tc.tile_pool(name="psS", bufs=2, space="PSUM") as psS,
    ):
        # ---- constants ----
        wq_sb = const.tile([128, KD, HPC * DH], BF16, name="wq_sb")
        wk_sb = const.tile([128, KD, HPC * DH], BF16, name="wk_sb")
        wv_sb = const.tile([128, KD, HPC * DH], BF16, name="wv_sb")
        wo_sb = const.tile([128, HPC, D], BF16, name="wo_sb")
        # cos/sin tables duplicated per head; sin signed [-sin; +sin]
        cs_sb = const.tile([128, HPC, L], BF16, name="cs_sb")
        sn_sb = const.tile([128, HPC, L], BF16, name="sn_sb")
        mk_sb = const.tile([128, DH], BF16, name="mk_sb")

        def load_chunk(b, c):
            c0 = c * TOKC
            xtile = xtp.tile([128, KD, TOKC], BF16, name="xtile")
            src = xt[b, :, c0:c0 + TOKC].rearrange("(ko p) n -> p ko n", p=128)
            # two half-loads: matmuls on k<8 start as soon as half 0 lands
            nc.sync.dma_start(xtile[:, 0:KD // 2], src[:, 0:KD // 2])
            nc.sync.dma_start(xtile[:, KD // 2:], src[:, KD // 2:])
            return xtile

        # first x chunk + projection weights ahead of everything else, in
        # consumption order (Q, K, V, RoPE table heads); mask/wo trail since
        # they are first needed well into the attention phase.
        if "qkv" in phases:
            xtile00 = xtp.tile([128, KD, TOKC], BF16, name="xtile")
            src00 = xt[0, :, 0:TOKC].rearrange("(ko p) n -> p ko n", p=128)
            for kq in range(4):
                nc.sync.dma_start(xtile00[:, 4 * kq:4 * (kq + 1)],
                                  src00[:, 4 * kq:4 * (kq + 1)])
                nc.sync.dma_start(wq_sb[:, 4 * kq:4 * (kq + 1)],
                                  wq[:, 4 * kq:4 * (kq + 1)])
        else:
            xtile00 = None
        nc.sync.dma_start(wk_sb, wk)
        nc.sync.dma_start(wv_sb, wv)
        nc.sync.dma_start(sn_sb[:, :, 0:2 * TOKC], sn[:, :, 0:2 * TOKC])
        nc.sync.dma_start(cs_sb[:, :, 0:2 * TOKC], cs[:, :, 0:2 * TOKC])

        def load_tail(c):
            # table tails and attention-phase constants interleave between
            # x-chunk loads so no critical load ever queues behind them
            if c == 2:
                nc.sync.dma_start(sn_sb[:, :, 2 * TOKC:], sn[:, :, 2 * TOKC:])
                nc.sync.dma_start(cs_sb[:, :, 2 * TOKC:], cs[:, :, 2 * TOKC:])
            elif c == 4:
                nc.sync.dma_start(mk_sb, mk)
                nc.sync.dma_start(wo_sb, wo)
        ones_f32 = const.tile([128, 1], F32, name="ones_f32")
        nc.vector.memset(ones_f32, 1.0)
        ones_col = const.tile([128, 1], BF16, name="ones_col")
        nc.vector.tensor_copy(ones_col, ones_f32)

        def emit_qkv(b):
            # qk[0] = q_rot, qk[1] = k_rot: [128 dh, HPC, L] bf16
            qrot = qkvp.tile([128, HPC, L], BF16, name="qrot")
            krot = qkvp.tile([128, HPC, L], BF16, name="krot")
            vnat = qkvp.tile([128, KT, HPC * DH], BF16, name="vnat")
            for c in range(NCHUNK):
                c0 = c * TOKC
                xtile = xtile00 if (b == 0 and c == 0) else load_chunk(b, c)
                if b == 0:
                    load_tail(c)
                for w_sb, dst in ((wq_sb, qrot), (wk_sb, krot)):
                    ps2 = psA.tile([128, HPC, TOKC], F32, name="ps_proj",
                                   tag="psA")
                    for h in range(HPC):
                        for k in range(KD):
                            nc.tensor.matmul(
                                ps2[:, h, :],
                                w_sb[:, k, h * DH:(h + 1) * DH],
                                xtile[:, k, :],
                                start=(k == 0), stop=(k == KD - 1),
                            )
                    # RoPE copyback: dst = ps*cos + swap_sign(ps)*sin
                    cseg = cs_sb[:, :, c0:c0 + TOKC]
                    sseg = sn_sb[:, :, c0:c0 + TOKC]
                    t = tmps.tile([128, HPC, TOKC], BF16, name="rope_t")
                    a = tmps.tile([128, HPC, TOKC], BF16, name="rope_a")
                    nc.vector.tensor_tensor(
                        t[0:64], ps2[64:128], sseg[0:64], ALU.mult)
                    nc.vector.tensor_tensor(
                        t[64:128], ps2[0:64], sseg[64:128], ALU.mult)
                    nc.vector.tensor_tensor(a, ps2, cseg, ALU.mult)
                    nc.vector.tensor_tensor(
                        dst[:, :, c0:c0 + TOKC], a, t, ALU.add)
                # V in natural [tok, dh] layout: x token slices stationary
                for s in range(TOKC // 128):
                    psv = psA.tile([128, HPC * DH], F32, name="ps_v",
                                   tag="psA")
                    for k in range(KD):
                        nc.tensor.matmul(
                            psv, xtile[:, k, s * 128:(s + 1) * 128],
                            wv_sb[:, k, :],
                            start=(k == 0), stop=(k == KD - 1),
                        )
                    kti = (c0 // 128) + s
                    nc.vector.tensor_copy(vnat[:, kti, :], psv)
            return qrot, krot, vnat

        def emit_wout(b, attn_sb, q0):
            for mt in range(QC // 128 if "wout" in phases else 0):
                t0 = q0 + mt * 128
                for ec in range(D // 512):
                    psw = psA.tile([128, 512], F32, name="ps_w", tag="psA")
                    for h in range(HPC):
                        nc.tensor.matmul(
                            psw, attn_sb[h][:, mt * 128:(mt + 1) * 128],
                            wo_sb[:, h, ec * 512:(ec + 1) * 512],
                            start=(h == 0), stop=(h == HPC - 1),
                        )
                    # copyback on DVE and DMA config on sync: keeps the ACT
                    # copyback on DVE: gpsimd cannot read PSUM (HW rule);
                    # ACT stays free for the exp chain
                    ob = outp.tile([128, 512], BF16, name="out_sb")
                    nc.vector.tensor_copy(ob, psw)
                    eng = nc.scalar if (mt + ec) % 2 == 0 else nc.sync
                    eng.dma_start(
                        out[b, t0:t0 + 128, ec * 512:(ec + 1) * 512], ob)

        def emit_attn(b, qrot, krot, vnat, interleave_wout):
            pend = None
            all_qc = []
            for qc in range(NQC):
                q0 = qc * QC
                attn_sb = []
                for h in range(HPC):
                    pso = psO.tile([128, QC], F32, name="ps_out")
                    pss = psS.tile([1, QC], F32, name="ps_sum")
                    nkt = (qc + 1) * (QC // 128)
                    etiles = {}
                    # scores run LAG tiles ahead of PV/sum on the PE queue so
                    # the exp(psc)->e latency hides behind later score matmuls
                    for kt in range(nkt + LAG):
                        if kt < nkt:
                            # diagonal blocks: columns below the diagonal are
                            # fully masked -- skip them in scores and exp
                            # (zeroed via idle gpsimd), and mask only the
                            # [128,128] triangle block
                            diag = kt - qc * (QC // 128)
                            lo = max(diag, 0) * 128
                            psc = psA.tile([128, QC], F32, name="ps_sc",
                                           tag="psA")
                            nc.tensor.matmul(
                                psc[:, lo:], krot[:, h, kt * 128:(kt + 1) * 128],
                                qrot[:, h, q0 + lo:q0 + QC],
                                start=True, stop=True,
                            )
                            e = esbp.tile([128, QC], BF16, name="e_sb")
                            if lo:
                                nc.vector.memset(e[:, 0:lo], 0.0)
                            nc.scalar.activation(e[:, lo:], psc[:, lo:],
                                                 AF.Exp, scale=SCALE)
                            if diag >= 0:
                                nc.vector.tensor_tensor(
                                    e[:, lo:lo + DH], e[:, lo:lo + DH],
                                    mk_sb, ALU.mult)
                            etiles[kt] = e
                        j = kt - LAG
                        if j >= 0:
                            e = etiles.pop(j)
                            nc.tensor.matmul(
                                pso, vnat[:, j, h * DH:(h + 1) * DH], e,
                                start=(j == 0), stop=(j == nkt - 1))
                            nc.tensor.matmul(pss, ones_col, e,
                                             start=(j == 0), stop=(j == nkt - 1))
                    att = attnp.tile([128, QC], BF16, name=f"att{h}")
                    rec = tmps.tile([1, QC], F32, name="recip")
                    nc.vector.reciprocal(rec, pss)
                    bc = bcp.tile([128, QC], F32, name="bc_sb")
                    nc.gpsimd.partition_broadcast(bc, rec)
                    nc.vector.tensor_tensor(att, pso, bc, ALU.mult)
                    attn_sb.append(att)
                if interleave_wout:
                    if pend is not None:
                        emit_wout(b, *pend)
                    pend = (attn_sb, q0)
                else:
                    all_qc.append((attn_sb, q0))
            if pend is not None:
                emit_wout(b, *pend)
            return all_qc

        prev = None
        for b in range(B):
            tiles_b = emit_qkv(b) if "qkv" in phases else None
            if prev is not None:
                pb, qcs = prev
                for attn_sb, q0 in qcs:
                    emit_wout(pb, attn_sb, q0)
                prev = None
            if "attn" in phases and tiles_b is not None:
                last = b == B - 1
                qcs = emit_attn(b, *tiles_b, interleave_wout=last)
                if not last:
                    prev = (b, qcs)


def build_kernel(timing=False, loop_n=0, phases=("qkv", "attn", "wout")):
    nc = bacc.Bacc(
        "TRN2",
        target_bir_lowering=False,
        debug=False,
        enable_asserts=False,
        num_devices=NCORES,
    )
    xt = nc.dram_tensor("xt", [B, D, L], BF16, kind="ExternalInput").ap()
    wq = nc.dram_tensor("wq", [128, KD, HPC * DH], BF16, kind="ExternalInput").ap()
    wk = nc.dram_tensor("wk", [128, KD, HPC * DH], BF16, kind="ExternalInput").ap()
    wv = nc.dram_tensor("wv", [128, KD, HPC * DH], BF16, kind="ExternalInput").ap()
    wo = nc.dram_tensor("wo", [128, HPC, D], BF16, kind="ExternalInput").ap()
    cs = nc.dram_tensor("cs", [128, HPC, L], BF16, kind="ExternalInput").ap()
    sn = nc.dram_tensor("sn", [128, HPC, L], BF16, kind="ExternalInput").ap()
    mk = nc.dram_tensor("mk", [128, DH], BF16, kind="ExternalInput").ap()
    out_kind = "Internal" if timing else "ExternalOutput"
    out = nc.dram_tensor("out", [B, L, D], BF16, kind=out_kind).ap()
    done = None
    if timing:
        done = nc.dram_tensor("done", [1, 4], BF16, kind="ExternalOutput").ap()

    nc.gpsimd.load_library(library_config.attn)
    aps = (xt, wq, wk, wv, wo, cs, sn, mk, out)
    with tile.TileContext(nc) as tc:
        if loop_n:
            with tc.For_i(0, loop_n, 1):
                _body(nc, tc, aps, phases)
        else:
            _body(nc, tc, aps, phases)
        if timing:
            # tiny output so the executable has an ExternalOutput; depends on
            # one real out tile via a DRAM->DRAM DMA of the last row.
            nc.sync.dma_start(done, out[B - 1, L - 1:L, 0:4])
    nc.compile()
    return nc


def _rope_tables():
    inv_freq = (1.0 / (ROPE_BASE ** (np.arange(0, DH, 2, dtype=np.float32) / DH))
                ).astype(np.float32)
    freqs = (np.arange(L, dtype=np.float32)[:, None] * inv_freq[None, :]
             ).astype(np.float32)  # [L, 64]
    cos_t = np.cos(freqs).T  # [64, L]
    sin_t = np.sin(freqs).T
    cos_full = np.concatenate([cos_t, cos_t], axis=0)  # [128, L]
    sin_full = np.concatenate([-sin_t, sin_t], axis=0)  # sign folded in
    # duplicate per head: [128, HPC, L]
    cs2 = np.ascontiguousarray(
        np.broadcast_to(cos_full[:, None, :], (128, HPC, L))
    ).astype(ml_dtypes.bfloat16)
    sn2 = np.ascontiguousarray(
        np.broadcast_to(sin_full[:, None, :], (128, HPC, L))
    ).astype(ml_dtypes.bfloat16)
    return cs2, sn2


def _host_inputs(x, w_qkv, w_out):
    xt = np.ascontiguousarray(np.transpose(x, (0, 2, 1))
                              ).astype(ml_dtypes.bfloat16)  # [B, D, L]
    cs2, sn2 = _rope_tables()
    p = np.arange(128)[:, None]
    f = np.arange(DH)[None, :]
    # triangle mask for the diagonal 128-col block: valid iff q_local >= k
    mk = np.ascontiguousarray(
        (p <= f).astype(np.float32)).astype(ml_dtypes.bfloat16)  # [128, 128]

    def wtile(wT):  # [D, M] -> [128, D//128, M]
        return np.ascontiguousarray(
            wT.reshape(KD, 128, wT.shape[1]).transpose(1, 0, 2)
        ).astype(ml_dtypes.bfloat16)

    in_maps = []
    for c in range(NCORES):
        r0 = c * HPC * DH
        r1 = r0 + HPC * DH
        wq_c = wtile(np.ascontiguousarray(w_qkv[r0:r1, :].T))
        wk_c = wtile(np.ascontiguousarray(w_qkv[D + r0:D + r1, :].T))
        wv_c = wtile(np.ascontiguousarray(w_qkv[2 * D + r0:2 * D + r1, :].T))
        wo_c = np.ascontiguousarray(
            w_out[:, r0:r1].T.reshape(HPC, 128, D).transpose(1, 0, 2)
        ).astype(ml_dtypes.bfloat16)
        in_maps.append({
            "xt": xt, "wq": wq_c, "wk": wk_c, "wv": wv_c, "wo": wo_c,
            "cs": cs2, "sn": sn2, "mk": mk,
        })
    return in_maps


_NC_CACHE = []


def _get_nc():
    if not _NC_CACHE:
        _NC_CACHE.append(build_kernel())
    return _NC_CACHE[0]


def kernel(x, w_qkv, w_out):
    x = np.asarray(x, dtype=np.float32)
    w_qkv = np.asarray(w_qkv, dtype=np.float32)
    w_out = np.asarray(w_out, dtype=np.float32)
    nc = _get_nc()
    in_maps = _host_inputs(x, w_qkv, w_out)
    res = run_bass_kernel_spmd(nc, in_maps, core_ids=list(range(NCORES)))
    acc = res.results[0]["out"].astype(np.float32)
    for c in range(1, NCORES):
        acc += res.results[c]["out"].astype(np.float32)
    return acc
